# revision 47
# baseline (speedup 1.0000x reference)
"""DrBC GNN message-passing kernel for 8 Trainium2 NeuronCores.

Sharding: nodes split contiguously across 8 cores. Edges bucketed by target
node's owner core and 128-node target tile. Per layer each core:
  1. dma_gather's source-node rows (bf16) from a replicated node table in HBM
     (split into lo/hi halves so indices fit int16),
  2. scatter-adds them into per-tile aggregates via PE matmuls against
     on-the-fly one-hot selection matrices (edge norm folded in),
  3. runs the GRU update (bias-opener matmuls + fused gi+gh accumulation),
  4. l2-normalizes, updates the running layer-max, and
  5. AllGathers the new shard into the next layer's replicated table
     (skipped after the last layer).
Decoder (2-layer MLP on the layer-max) runs on the local shard only.
"""

import numpy as np
import ml_dtypes

import concourse.bass as bass
import concourse.bacc as bacc
import concourse.mybir as mybir
from concourse.tile import TileContext
from concourse.bass_utils import run_bass_kernel_spmd

F32 = mybir.dt.float32
BF16 = mybir.dt.bfloat16
I16 = mybir.dt.int16
I32 = mybir.dt.int32
AF = mybir.ActivationFunctionType
ALU = mybir.AluOpType

EPS = 1e-12


class Cfg:
    def __init__(self, N=50000, E=600000, L=5, n_cores=8, group_tiles=4,
                 single_packet=False):
        self.N, self.E, self.L, self.n_cores = N, E, L, n_cores
        self.H = 128
        self.IN = 3
        self.HID = 64
        assert N % n_cores == 0
        self.NSH = N // n_cores              # nodes per core
        self.NT = (self.NSH + 127) // 128    # node tiles per core
        self.NSH_PAD = self.NT * 128
        self.SPLIT = (N // 2 + 127) // 128 * 128  # lo/hi table split (int16 idx)
        assert self.SPLIT < 32768 and N - self.SPLIT < 32768
        self.GT = group_tiles                # tiles per gather group
        self.single_packet = single_packet


def build_plan(cfg, edge_idx):
    """Bucket edges by (core, target tile), pad chunk counts uniformly across
    cores (SPMD: one NEFF), build per-core gather-index/chunk-meta arrays."""
    row = np.asarray(edge_idx[0], dtype=np.int64)
    col = np.asarray(edge_idx[1], dtype=np.int64)
    N = cfg.N
    deg = np.bincount(col, minlength=N).astype(np.float64) + 1.0
    dinv = deg ** -0.5
    norm = (dinv[row] * dinv[col]).astype(np.float32)

    n_cores, NT = cfg.n_cores, cfg.NT
    core_of = col // cfg.NSH
    tile_of = (col % cfg.NSH) // 128
    v_of = ((col % cfg.NSH) % 128).astype(np.float32)
    is_hi = (row >= cfg.SPLIT).astype(np.int64)
    rloc = np.where(is_hi == 1, row - cfg.SPLIT, row).astype(np.int16)

    # bucket key: (core, tile, half)
    key = (core_of * NT + tile_of) * 2 + is_hi
    nkey = n_cores * NT * 2
    order = np.argsort(key, kind="stable")
    ks, rs, vs, ns = key[order], rloc[order], v_of[order], norm[order]
    counts = np.bincount(ks, minlength=nkey)
    starts = np.concatenate([[0], np.cumsum(counts)[:-1]])

    cnt = counts.reshape(n_cores, NT, 2)
    # uniform per-tile chunk counts = max over cores (>=1 so psum groups exist)
    nlo = np.maximum((cnt[:, :, 0].max(axis=0) + 127) // 128, 1)
    nhi = np.maximum((cnt[:, :, 1].max(axis=0) + 127) // 128, 1)
    nch2 = np.stack([nlo, nhi], axis=1)          # [NT, 2]

    groups = [list(range(t0, min(t0 + cfg.GT, NT))) for t0 in range(0, NT, cfg.GT)]
    NCHUNK = int(nlo.sum() + nhi.sum())
    W_tot = 8 * NCHUNK

    # padded flat layout, order (t, half) shared by all cores
    pad_off = np.zeros((NT, 2), dtype=np.int64)
    po = 0
    for t in range(NT):
        for h in (0, 1):
            pad_off[t, h] = po
            po += int(nch2[t, h]) * 128
    PADTOT = po

    # scatter edges into per-core padded arrays; pad slots keep idx=-1 so the
    # gather firmware trims them (they sit at each bucket's tail per core)
    idxpad = np.full((n_cores, PADTOT), -1, dtype=np.int16)
    vpad = np.zeros((n_cores, PADTOT), dtype=np.float32)
    npad = np.zeros((n_cores, PADTOT), dtype=np.float32)
    rank = np.arange(len(ks)) - starts[ks]
    kc, kt, kh = ks // (NT * 2), (ks // 2) % NT, ks % 2
    dest = pad_off[kt, kh] + rank
    idxpad[kc, dest] = rs
    vpad[kc, dest] = vs
    npad[kc, dest] = ns

    # chunk ids in emission order: per group, per tile: lo chunks then hi chunks
    chunk_id = {}
    cid = 0
    for ts in groups:
        for t in ts:
            for j in range(int(nlo[t])):
                chunk_id[(t, 0, j)] = cid; cid += 1
            for j in range(int(nhi[t])):
                chunk_id[(t, 1, j)] = cid; cid += 1
    assert cid == NCHUNK

    # gather-call layout: one call per (tile, half) bucket so each call's
    # trailing -1 pads are trimmed by the firmware. Buckets of a group are
    # laid out consecutively in gidx; buffer positions cover the group.
    call_off = []          # per group: (lo_n, hi_n) buffer chunk counts
    buckets = []           # per group: list of (t, half, gidx_col_off, n_b, buf_chunk0)
    buf_pos = {}
    col_cursor = 0
    for ts in groups:
        lo_n = int(sum(nlo[t] for t in ts))
        hi_n = int(sum(nhi[t] for t in ts))
        call_off.append((lo_n, hi_n))
        blist = []
        p = 0
        for t in ts:
            blist.append((t, 0, col_cursor, int(nlo[t]), p))
            col_cursor += 8 * int(nlo[t])
            for j in range(int(nlo[t])):
                buf_pos[(t, 0, j)] = p; p += 1
        p = 0
        for t in ts:
            blist.append((t, 1, col_cursor, int(nhi[t]), p))
            col_cursor += 8 * int(nhi[t])
            for j in range(int(nhi[t])):
                buf_pos[(t, 1, j)] = p; p += 1
        buckets.append(blist)
    assert col_cursor == W_tot

    gidx = np.zeros((n_cores, 128, W_tot), dtype=np.int16)
    for c in range(n_cores):
        for blist in buckets:
            for (t, half, off, n_b, _p0) in blist:
                sl = idxpad[c, pad_off[t, half]: pad_off[t, half] + n_b * 128]
                wrapped = sl.reshape(-1, 16).T
                gidx[c, :, off:off + wrapped.shape[1]] = np.tile(wrapped, (8, 1))

    # host-precomputed one-hot scatter matrices: S[p, cid*128 + v] = norm of the
    # edge in slot p of chunk cid targeting within-tile node v (0 for pad slots)
    bf = ml_dtypes.bfloat16
    S_full = np.zeros((n_cores, 128, NCHUNK * 128), dtype=bf)
    for c in range(n_cores):
        for t in range(NT):
            for half, nch_arr in ((0, nlo), (1, nhi)):
                c0 = chunk_id[(t, half, 0)]
                n_ = int(nch_arr[t])
                sl = slice(pad_off[t, half], pad_off[t, half] + n_ * 128)
                v = vpad[c, sl].astype(np.int64).reshape(n_, 128)   # [chunk, p]
                nval = npad[c, sl].reshape(n_, 128)
                for j in range(n_):
                    S_full[c, np.arange(128), (c0 + j) * 128 + v[j]] = \
                        nval[j].astype(bf)

    # exact per-core edge count per (tile, half) bucket -> num_idxs_reg
    bcnt = np.ascontiguousarray(
        cnt.reshape(n_cores, NT * 2).astype(np.int32))

    return dict(nlo=nlo, nhi=nhi, groups=groups, NCHUNK=NCHUNK, W_tot=W_tot,
                call_off=call_off, buckets=buckets, chunk_id=chunk_id,
                buf_pos=buf_pos, S_full=S_full, bcnt=bcnt), gidx


def build_nc(cfg, plan, b_out_val):
    nc = bacc.Bacc("TRN2", target_bir_lowering=False, debug=False,
                   num_devices=cfg.n_cores, num_swdge_queues=4,
                   dynamic_dma_scratch_size=32768)
    H, NT, NSH, L = cfg.H, cfg.NT, cfg.NSH, cfg.L
    NCHUNK = plan["NCHUNK"]
    nlo, nhi, groups = plan["nlo"], plan["nhi"], plan["groups"]
    call_off, chunk_id, buf_pos = plan["call_off"], plan["chunk_id"], plan["buf_pos"]
    buckets = plan["buckets"]
    maxlo = max(lo for lo, hi in call_off)
    maxhi = max(hi for lo, hi in call_off)
    RG = [list(range(cfg.n_cores))]
    last_rows = NSH - (NT - 1) * 128

    p_gidx = nc.declare_dram_parameter("gidx", [128, plan["W_tot"]], I16, isOutput=False)
    p_bcnt = nc.declare_dram_parameter("bcnt", [1, NT * 2], I32, isOutput=False)
    p_S = nc.declare_dram_parameter("S_full", [128, NCHUNK * 128], BF16, isOutput=False)
    p_xT = nc.declare_dram_parameter("xT", [cfg.IN, cfg.NSH_PAD], F32, isOutput=False)
    p_ident = nc.declare_dram_parameter("ident", [128, 128], BF16, isOutput=False)
    p_wemb = nc.declare_dram_parameter("wembT", [cfg.IN, H], F32, isOutput=False)
    p_bemb = nc.declare_dram_parameter("bemb", [1, H], F32, isOutput=False)
    p_wih_rz = nc.declare_dram_parameter("wih_rz", [H, L * 256], BF16, isOutput=False)
    p_wih_n = nc.declare_dram_parameter("wih_n", [H, L * 128], BF16, isOutput=False)
    p_whh_rz = nc.declare_dram_parameter("whh_rz", [H, L * 256], BF16, isOutput=False)
    p_whh_n = nc.declare_dram_parameter("whh_n", [H, L * 128], BF16, isOutput=False)
    p_ball = nc.declare_dram_parameter("ball", [1, L * 512], F32, isOutput=False)
    p_whid = nc.declare_dram_parameter("whidT", [H, cfg.HID], BF16, isOutput=False)
    p_bhid = nc.declare_dram_parameter("bhid", [cfg.HID, 1], F32, isOutput=False)
    p_wout = nc.declare_dram_parameter("woutT", [cfg.HID, 1], BF16, isOutput=False)
    p_ones = nc.declare_dram_parameter("ones1", [1, 128], F32, isOutput=False)
    p_out = nc.declare_dram_parameter("out", [NSH, 1], F32, isOutput=True)

    tabA = nc.dram_tensor("tabA", [cfg.N, H], BF16, addr_space="Shared")
    tabB = nc.dram_tensor("tabB", [cfg.N, H], BF16, addr_space="Shared")
    shard_out = nc.dram_tensor("shard_out", [NSH, H], BF16)

    with TileContext(nc) as tc:
        with (
            tc.tile_pool(name="consts", bufs=1) as cpool,
            tc.tile_pool(name="zpool", bufs=1) as zpool,
            tc.tile_pool(name="work", bufs=3) as wpool,
            tc.tile_pool(name="spool", bufs=2) as spool,
            tc.tile_pool(name="gw", bufs=2) as gw,
            tc.tile_pool(name="psA", bufs=2, space="PSUM") as ppool,
            tc.tile_pool(name="psB", bufs=2, space="PSUM") as qpool,
            tc.tile_pool(name="psT", bufs=2, space="PSUM") as tpool,
        ):
            def load_const(ap, dtype, tag):
                t = cpool.tile(list(ap.shape), dtype, tag=tag)
                nc.sync.dma_start(out=t[:], in_=ap[:])
                return t

            gidx_t = load_const(p_gidx, I16, "gidx")
            bcnt_t = load_const(p_bcnt, I32, "bcnt")
            xT_t = load_const(p_xT, F32, "xT")
            ident_t = load_const(p_ident, BF16, "ident")
            wemb_t = load_const(p_wemb, F32, "wemb")
            bemb_t = load_const(p_bemb, F32, "bemb")
            wih_rz_t = load_const(p_wih_rz, BF16, "wihrz")
            wih_n_t = load_const(p_wih_n, BF16, "wihn")
            whh_rz_t = load_const(p_whh_rz, BF16, "whhrz")
            whh_n_t = load_const(p_whh_n, BF16, "whhn")
            ball_t = load_const(p_ball, F32, "ball")
            whid_t = load_const(p_whid, BF16, "whid")
            bhid_t = load_const(p_bhid, F32, "bhid")
            wout_t = load_const(p_wout, BF16, "wout")
            ones_t = load_const(p_ones, F32, "ones")

            bcnt_reg = nc.gpsimd.alloc_register("bcnt_reg")
            zmax_t = zpool.tile([128, cfg.NSH_PAD], BF16, tag="zmax")
            h_shA = zpool.tile([128, cfg.NSH_PAD], BF16, tag="hshA")
            h_shB = zpool.tile([128, cfg.NSH_PAD], BF16, tag="hshB")
            h_bufs = [h_shA, h_shB]
            # fixed ping-pong gather buffers (memset once: -1-trimmed pad
            # slots keep stale-but-finite data, zeroed by the S matmul)
            glo_bufs = [zpool.tile([128, maxlo * 128], BF16, tag=f"glo{i}",
                                   name=f"glo{i}") for i in range(2)]
            ghi_bufs = [zpool.tile([128, maxhi * 128], BF16, tag=f"ghi{i}",
                                   name=f"ghi{i}") for i in range(2)]
            ones_f = zpool.tile([128, 128], F32, tag="onesf")
            nc.vector.memset(ones_f[:], 1.0)
            eps_f = zpool.tile([128, 8], F32, tag="epsf")
            nc.vector.memset(eps_f[:], EPS)
            for b in glo_bufs + ghi_bufs:
                nc.vector.memset(b[:], 0)

            def rows_of(t):
                return 128 if t < NT - 1 else last_rows

            def l2norm_bf(pre_t, out_ap):
                """l2-normalize pre_t [128,H] f32 -> bf16 out_ap."""
                sq = wpool.tile([128, H], F32, tag="sq")
                ss = wpool.tile([128, 1], F32, tag="ss")
                nc.scalar.activation(sq[:], pre_t[:], AF.Square, accum_out=ss[:])
                s1 = wpool.tile([128, 1], F32, tag="s1")
                nc.scalar.activation(s1[:], ss[:], AF.Sqrt)
                s2 = wpool.tile([128, 1], F32, tag="s2")
                nc.vector.tensor_tensor(s2[:], s1[:], eps_f[:, 0:1], ALU.max)
                rec = wpool.tile([128, 1], F32, tag="rec")
                nc.vector.reciprocal(rec[:], s2[:])
                nc.vector.tensor_tensor(out_ap, pre_t[:],
                                        rec[:].broadcast_to((128, H)), ALU.mult)

            # ================= EMBED =================
            for t in range(NT):
                ps = ppool.tile([128, H], F32, tag="acc")
                nc.tensor.matmul(ps[:], ones_t[:], bemb_t[:], start=True, stop=False)
                nc.tensor.matmul(ps[:], xT_t[:, t * 128:(t + 1) * 128], wemb_t[:],
                                 start=False, stop=True)
                h0f = wpool.tile([128, H], F32, tag="pref")
                nc.scalar.activation(h0f[:], ps[:], AF.Relu)
                hsl = h_bufs[0][:, t * 128:(t + 1) * 128]
                l2norm_bf(h0f, hsl)
                nc.vector.tensor_copy(zmax_t[:, t * 128:(t + 1) * 128], hsl)
                r = rows_of(t)
                nc.sync.dma_start(out=shard_out[t * 128: t * 128 + r, :],
                                  in_=h_bufs[0][:r, t * 128: t * 128 + H])
            nc.gpsimd.collective_compute(
                "AllGather", ALU.bypass, replica_groups=RG,
                ins=[shard_out[:]], outs=[tabA[:]],
            )

            # ================= GRU LAYERS =================
            for l in range(L):
                tab_prev = tabA if l % 2 == 0 else tabB
                tab_cur = tabB if l % 2 == 0 else tabA
                qrot = 0
                for g_, ts in enumerate(groups):
                    cid_g0 = chunk_id[(ts[0], 0, 0)]
                    gch = int(sum(nlo[t] + nhi[t] for t in ts))
                    Sg = spool.tile([128, gch * 128], BF16, tag="S")
                    nc.sync.dma_start(
                        out=Sg[:],
                        in_=p_S[:, cid_g0 * 128:(cid_g0 + gch) * 128])
                    glo = glo_bufs[g_ % 2]
                    ghi = ghi_bufs[g_ % 2]
                    for (t, half, off, n_b, p0) in buckets[g_]:
                        if n_b == 0:
                            continue
                        buf = glo if half == 0 else ghi
                        tab_lo = 0 if half == 0 else cfg.SPLIT
                        tab_hi = cfg.SPLIT if half == 0 else cfg.N
                        nc.gpsimd.reg_load(
                            bcnt_reg,
                            bcnt_t[0:1, 2 * t + half: 2 * t + half + 1])
                        nc.gpsimd.dma_gather(
                            buf[:, p0 * 128:(p0 + n_b) * 128]
                                .rearrange("p (c e) -> p c e", e=128),
                            tab_prev[tab_lo:tab_hi, :],
                            gidx_t[:, off: off + 8 * n_b],
                            n_b * 128, bcnt_reg, H,
                            single_packet=cfg.single_packet,
                            queue_num=qrot % 4,
                        )
                        qrot += 1
                    h_prev = h_bufs[l % 2]
                    h_next = h_bufs[(l + 1) % 2]
                    NTS = len(ts)
                    td = [dict(t=t, base=t * 128) for t in ts]
                    # S0: hT transposes (PE) + copies (scalar, Copy table)
                    for i, d_ in enumerate(td):
                        tps = tpool.tile([128, H], BF16, tag="tps")
                        nc.tensor.transpose(
                            tps[:], h_prev[:, d_["base"]:d_["base"] + H],
                            ident_t[:])
                        d_["hT"] = gw.tile([128, H], BF16, tag=f"hT{i}",
                                           name=f"hT{i}")
                        nc.scalar.activation(d_["hT"][:], tps[:], AF.Copy)
                    # S1: scatter matmul chains + aggT copies
                    for i, d_ in enumerate(td):
                        t = d_["t"]
                        nch = int(nlo[t] + nhi[t])
                        sbase = chunk_id[(t, 0, 0)] - cid_g0
                        aggT_ps = ppool.tile([128, H], F32, tag="acc")
                        for k in range(nch):
                            if k < int(nlo[t]):
                                buf, bp = glo, buf_pos[(t, 0, k)]
                            else:
                                buf, bp = ghi, buf_pos[(t, 1, k - int(nlo[t]))]
                            nc.tensor.matmul(
                                aggT_ps[:],
                                buf[:, bp * 128:(bp + 1) * 128],
                                Sg[:, (sbase + k) * 128:(sbase + k + 1) * 128],
                                start=(k == 0), stop=(k == nch - 1),
                            )
                        d_["aggT"] = gw.tile([128, H], BF16, tag=f"aggT{i}",
                                             name=f"aggT{i}")
                        nc.scalar.activation(d_["aggT"][:], aggT_ps[:], AF.Copy)
                    # S2: gate PSUM bias init (scalar Copy) + gate matmuls (PE)
                    for i, d_ in enumerate(td):
                        gall = qpool.tile([128, 512], F32, tag=f"gall{i}",
                                          name=f"gall{i}", bufs=1)
                        d_["gall"] = gall
                        nc.tensor.matmul(gall[:, 0:256], ones_t[:],
                                         ball_t[:, l * 512:l * 512 + 256],
                                         start=True, stop=False)
                        nc.tensor.matmul(gall[:, 0:256], d_["aggT"][:],
                                         wih_rz_t[:, l * 256:(l + 1) * 256],
                                         start=False, stop=False)
                        nc.tensor.matmul(gall[:, 0:256], d_["hT"][:],
                                         whh_rz_t[:, l * 256:(l + 1) * 256],
                                         start=False, stop=True)
                        nc.tensor.matmul(gall[:, 256:384], ones_t[:],
                                         ball_t[:, l * 512 + 256:l * 512 + 384],
                                         start=True, stop=False)
                        nc.tensor.matmul(gall[:, 256:384], d_["aggT"][:],
                                         wih_n_t[:, l * 128:(l + 1) * 128],
                                         start=False, stop=True)
                        nc.tensor.matmul(gall[:, 384:512], ones_t[:],
                                         ball_t[:, l * 512 + 384:(l + 1) * 512],
                                         start=True, stop=False)
                        nc.tensor.matmul(gall[:, 384:512], d_["hT"][:],
                                         whh_n_t[:, l * 128:(l + 1) * 128],
                                         start=False, stop=True)
                    # S3: sigmoid batch (r, z gates)
                    for i, d_ in enumerate(td):
                        d_["rzt"] = gw.tile([128, 256], F32, tag=f"rzt{i}",
                                            name=f"rzt{i}")
                        nc.scalar.activation(d_["rzt"][:],
                                             d_["gall"][:, 0:256], AF.Sigmoid)
                    # S4: candidate pre-activation (DVE)
                    for i, d_ in enumerate(td):
                        t1 = wpool.tile([128, H], F32, tag="t1")
                        nc.vector.tensor_mul(t1[:], d_["rzt"][:, 0:128],
                                             d_["gall"][:, 384:512])
                        d_["t2"] = gw.tile([128, H], F32, tag=f"t2{i}",
                                           name=f"t2{i}")
                        nc.vector.tensor_add(d_["t2"][:], t1[:],
                                             d_["gall"][:, 256:384])
                    # S5: sigmoid batch #2: tanh(x) = 2*sigmoid(2x) - 1
                    for i, d_ in enumerate(td):
                        d_["sg2"] = gw.tile([128, H], F32, tag=f"sg2{i}",
                                            name=f"sg2{i}")
                        nc.scalar.activation(d_["sg2"][:], d_["t2"][:],
                                             AF.Sigmoid, scale=2.0)
                    # S6: GRU blend (DVE)
                    for i, d_ in enumerate(td):
                        hp_sl = h_prev[:, d_["base"]:d_["base"] + H]
                        ng = wpool.tile([128, H], F32, tag="ng")
                        nc.vector.scalar_tensor_tensor(
                            ng[:], d_["sg2"][:], 2.0, ones_f[:],
                            ALU.mult, ALU.subtract)
                        dd = wpool.tile([128, H], F32, tag="d")
                        nc.vector.tensor_sub(dd[:], hp_sl, ng[:])
                        e = wpool.tile([128, H], F32, tag="e")
                        nc.vector.tensor_mul(e[:], dd[:], d_["rzt"][:, 128:256])
                        d_["pre"] = gw.tile([128, H], F32, tag=f"pre{i}",
                                            name=f"pre{i}")
                        nc.vector.tensor_add(d_["pre"][:], e[:], ng[:])
                    # S7: squares batch (scalar) into grouped accum columns
                    ssg = gw.tile([128, NTS], F32, tag="ssg")
                    for i, d_ in enumerate(td):
                        sq = wpool.tile([128, H], F32, tag="sq")
                        nc.scalar.activation(sq[:], d_["pre"][:], AF.Square,
                                             accum_out=ssg[:, i:i + 1])
                    # S8: batched sqrt + guard + reciprocal
                    s1g = gw.tile([128, NTS], F32, tag="s1g")
                    nc.scalar.activation(s1g[:], ssg[:], AF.Sqrt)
                    s2g = gw.tile([128, NTS], F32, tag="s2g")
                    nc.vector.tensor_tensor(s2g[:], s1g[:], eps_f[:, 0:NTS],
                                            ALU.max)
                    recg = gw.tile([128, NTS], F32, tag="recg")
                    nc.vector.reciprocal(recg[:], s2g[:])
                    # S9: normalize, layer-max, shard write
                    for i, d_ in enumerate(td):
                        base = d_["base"]
                        hn_sl = h_next[:, base:base + H]
                        nc.vector.tensor_tensor(
                            hn_sl, d_["pre"][:],
                            recg[:, i:i + 1].broadcast_to((128, H)), ALU.mult)
                        nc.vector.tensor_max(zmax_t[:, base:base + 128],
                                             zmax_t[:, base:base + 128], hn_sl)
                        if l < L - 1:
                            r = rows_of(d_["t"])
                            nc.sync.dma_start(
                                out=shard_out[base: base + r, :],
                                in_=h_next[:r, base:base + H])
                if l < L - 1:
                    nc.gpsimd.collective_compute(
                        "AllGather", ALU.bypass, replica_groups=RG,
                        ins=[shard_out[:]], outs=[tab_cur[:]],
                    )

            # ================= DECODER =================
            for t in range(NT):
                r = rows_of(t)
                base = t * 128
                tps = tpool.tile([128, H], BF16, tag="tps")
                nc.tensor.transpose(tps[:], zmax_t[:, base:base + 128], ident_t[:])
                zT = wpool.tile([128, H], BF16, tag="zT")
                nc.vector.tensor_copy(zT[:], tps[:])
                galld = qpool.tile([128, 512], F32, tag="gall0",
                                   name="galld", bufs=1)
                hid_ps = galld[0:cfg.HID, 0:128]
                nc.tensor.matmul(hid_ps, whid_t[:], zT[:], start=True, stop=True)
                hid = wpool.tile([cfg.HID, 128], BF16, tag="hid")
                nc.scalar.activation(hid[:], hid_ps, AF.Relu, bias=bhid_t[:])
                o_ps = galld[0:1, 256:384]
                nc.tensor.matmul(o_ps, wout_t[:], hid[:], start=True, stop=True)
                o_sb = wpool.tile([1, 128], F32, tag="osb")
                nc.scalar.activation(o_sb[:], o_ps, AF.Copy, bias=float(b_out_val))
                nc.sync.dma_start(out=p_out[base: base + r, :], in_=o_sb[:1, :r])
    nc.compile()
    return nc


def make_in_maps(cfg, inputs, plan, gidx):
    bf = ml_dtypes.bfloat16
    L, H, NSH = cfg.L, cfg.H, cfg.NSH
    x = np.asarray(inputs["x"], np.float32)
    w_ih = np.asarray(inputs["w_ih"], np.float32)
    w_hh = np.asarray(inputs["w_hh"], np.float32)
    b_ih = np.asarray(inputs["b_ih"], np.float32)
    b_hh = np.asarray(inputs["b_hh"], np.float32)

    wih_rz = np.concatenate([w_ih[l, :256, :].T for l in range(L)], axis=1)
    wih_n = np.concatenate([w_ih[l, 256:384, :].T for l in range(L)], axis=1)
    whh_rz = np.concatenate([w_hh[l, :256, :].T for l in range(L)], axis=1)
    whh_n = np.concatenate([w_hh[l, 256:384, :].T for l in range(L)], axis=1)
    ball = np.concatenate(
        [np.concatenate([b_ih[l, :256] + b_hh[l, :256],
                         b_ih[l, 256:384], b_hh[l, 256:384]])
         for l in range(L)])[None, :]

    common = {
        "ident": np.eye(128, dtype=bf),
        "wembT": np.ascontiguousarray(np.asarray(inputs["W_embed"], np.float32).T),
        "bemb": np.asarray(inputs["b_embed"], np.float32)[None, :],
        "wih_rz": np.ascontiguousarray(wih_rz, dtype=bf),
        "wih_n": np.ascontiguousarray(wih_n, dtype=bf),
        "whh_rz": np.ascontiguousarray(whh_rz, dtype=bf),
        "whh_n": np.ascontiguousarray(whh_n, dtype=bf),
        "ball": np.ascontiguousarray(ball),
        "whidT": np.ascontiguousarray(np.asarray(inputs["W_hid"], np.float32).T,
                                      dtype=bf),
        "bhid": np.asarray(inputs["b_hid"], np.float32)[:, None],
        "woutT": np.ascontiguousarray(np.asarray(inputs["W_out"], np.float32).T,
                                      dtype=bf),
        "ones1": np.ones((1, 128), np.float32),
    }
    in_maps = []
    for c in range(cfg.n_cores):
        xT = np.zeros((cfg.IN, cfg.NSH_PAD), np.float32)
        xT[:, :NSH] = x[c * NSH:(c + 1) * NSH, :].T
        m = dict(common)
        m["xT"] = xT
        m["gidx"] = gidx[c]
        m["S_full"] = plan["S_full"][c]
        m["bcnt"] = plan["bcnt"][c][None, :]
        in_maps.append(m)
    return in_maps


def kernel(**inputs):
    cfg = Cfg()
    plan, gidx = build_plan(cfg, np.asarray(inputs["edge_idx"]))
    nc = build_nc(cfg, plan, float(np.asarray(inputs["b_out"]).ravel()[0]))
    in_maps = make_in_maps(cfg, inputs, plan, gidx)
    res = run_bass_kernel_spmd(nc, in_maps, list(range(cfg.n_cores)))
    out = np.concatenate([res.results[c]["out"] for c in range(cfg.n_cores)], axis=0)
    return out.astype(np.float32)



# revision 49
# speedup vs baseline: 1.0335x; 1.0335x over previous
"""DrBC GNN message-passing kernel for 8 Trainium2 NeuronCores.

Sharding: nodes split contiguously across 8 cores. Edges bucketed by target
node's owner core and 128-node target tile. Per layer each core:
  1. dma_gather's source-node rows (bf16) from a replicated node table in HBM
     (split into lo/hi halves so indices fit int16),
  2. scatter-adds them into per-tile aggregates via PE matmuls against
     on-the-fly one-hot selection matrices (edge norm folded in),
  3. runs the GRU update (bias-opener matmuls + fused gi+gh accumulation),
  4. l2-normalizes, updates the running layer-max, and
  5. AllGathers the new shard into the next layer's replicated table
     (skipped after the last layer).
Decoder (2-layer MLP on the layer-max) runs on the local shard only.
"""

import numpy as np
import ml_dtypes

import concourse.bass as bass
import concourse.bacc as bacc
import concourse.mybir as mybir
from concourse.tile import TileContext
from concourse.bass_utils import run_bass_kernel_spmd

F32 = mybir.dt.float32
BF16 = mybir.dt.bfloat16
I16 = mybir.dt.int16
I32 = mybir.dt.int32
AF = mybir.ActivationFunctionType
ALU = mybir.AluOpType

EPS = 1e-12


class Cfg:
    def __init__(self, N=50000, E=600000, L=5, n_cores=8, group_tiles=4,
                 single_packet=False):
        self.N, self.E, self.L, self.n_cores = N, E, L, n_cores
        self.H = 128
        self.IN = 3
        self.HID = 64
        assert N % n_cores == 0
        self.NSH = N // n_cores              # nodes per core
        self.NT = (self.NSH + 127) // 128    # node tiles per core
        self.NSH_PAD = self.NT * 128
        self.T1 = 28 * 128                   # shard split: rows [0,T1) -> table1
        assert n_cores * self.T1 < 32768
        assert n_cores * (self.NSH - self.T1) < 32768
        self.GT = group_tiles                # tiles per gather group
        self.single_packet = single_packet


def build_plan(cfg, edge_idx):
    """Bucket edges by (core, target tile), pad chunk counts uniformly across
    cores (SPMD: one NEFF), build per-core gather-index/chunk-meta arrays."""
    row = np.asarray(edge_idx[0], dtype=np.int64)
    col = np.asarray(edge_idx[1], dtype=np.int64)
    N = cfg.N
    deg = np.bincount(col, minlength=N).astype(np.float64) + 1.0
    dinv = deg ** -0.5
    norm = (dinv[row] * dinv[col]).astype(np.float32)

    n_cores, NT = cfg.n_cores, cfg.NT
    core_of = col // cfg.NSH
    tile_of = (col % cfg.NSH) // 128
    v_of = ((col % cfg.NSH) % 128).astype(np.float32)
    src_core = row // cfg.NSH
    src_loc = row % cfg.NSH
    T1, T2 = cfg.T1, cfg.NSH - cfg.T1
    is_hi = (src_loc >= T1).astype(np.int64)
    rloc = np.where(is_hi == 1, src_core * T2 + (src_loc - T1),
                    src_core * T1 + src_loc).astype(np.int16)

    # bucket key: (core, tile, half)
    key = (core_of * NT + tile_of) * 2 + is_hi
    nkey = n_cores * NT * 2
    order = np.argsort(key, kind="stable")
    ks, rs, vs, ns = key[order], rloc[order], v_of[order], norm[order]
    counts = np.bincount(ks, minlength=nkey)
    starts = np.concatenate([[0], np.cumsum(counts)[:-1]])

    cnt = counts.reshape(n_cores, NT, 2)
    # uniform per-tile chunk counts = max over cores (>=1 so psum groups exist)
    nlo = np.maximum((cnt[:, :, 0].max(axis=0) + 127) // 128, 1)
    nhi = np.maximum((cnt[:, :, 1].max(axis=0) + 127) // 128, 1)
    nch2 = np.stack([nlo, nhi], axis=1)          # [NT, 2]

    groups = [list(range(t0, min(t0 + cfg.GT, NT))) for t0 in range(0, NT, cfg.GT)]
    NCHUNK = int(nlo.sum() + nhi.sum())
    W_tot = 8 * NCHUNK

    # padded flat layout, order (t, half) shared by all cores
    pad_off = np.zeros((NT, 2), dtype=np.int64)
    po = 0
    for t in range(NT):
        for h in (0, 1):
            pad_off[t, h] = po
            po += int(nch2[t, h]) * 128
    PADTOT = po

    # scatter edges into per-core padded arrays; pad slots keep idx=-1 so the
    # gather firmware trims them (they sit at each bucket's tail per core)
    idxpad = np.full((n_cores, PADTOT), -1, dtype=np.int16)
    vpad = np.zeros((n_cores, PADTOT), dtype=np.float32)
    npad = np.zeros((n_cores, PADTOT), dtype=np.float32)
    rank = np.arange(len(ks)) - starts[ks]
    kc, kt, kh = ks // (NT * 2), (ks // 2) % NT, ks % 2
    dest = pad_off[kt, kh] + rank
    idxpad[kc, dest] = rs
    vpad[kc, dest] = vs
    npad[kc, dest] = ns

    # chunk ids in emission order: per group, per tile: lo chunks then hi chunks
    chunk_id = {}
    cid = 0
    for ts in groups:
        for t in ts:
            for j in range(int(nlo[t])):
                chunk_id[(t, 0, j)] = cid; cid += 1
            for j in range(int(nhi[t])):
                chunk_id[(t, 1, j)] = cid; cid += 1
    assert cid == NCHUNK

    # gather-call layout: one call per (tile, half) bucket so each call's
    # trailing -1 pads are trimmed by the firmware. Buckets of a group are
    # laid out consecutively in gidx; buffer positions cover the group.
    call_off = []          # per group: (lo_n, hi_n) buffer chunk counts
    buckets = []           # per group: list of (t, half, gidx_col_off, n_b, buf_chunk0)
    buf_pos = {}
    col_cursor = 0
    for ts in groups:
        lo_n = int(sum(nlo[t] for t in ts))
        hi_n = int(sum(nhi[t] for t in ts))
        call_off.append((lo_n, hi_n))
        blist = []
        p = 0
        for t in ts:
            blist.append((t, 0, col_cursor, int(nlo[t]), p))
            col_cursor += 8 * int(nlo[t])
            for j in range(int(nlo[t])):
                buf_pos[(t, 0, j)] = p; p += 1
        p = 0
        for t in ts:
            blist.append((t, 1, col_cursor, int(nhi[t]), p))
            col_cursor += 8 * int(nhi[t])
            for j in range(int(nhi[t])):
                buf_pos[(t, 1, j)] = p; p += 1
        buckets.append(blist)
    assert col_cursor == W_tot

    gidx = np.zeros((n_cores, 128, W_tot), dtype=np.int16)
    for c in range(n_cores):
        for blist in buckets:
            for (t, half, off, n_b, _p0) in blist:
                sl = idxpad[c, pad_off[t, half]: pad_off[t, half] + n_b * 128]
                wrapped = sl.reshape(-1, 16).T
                gidx[c, :, off:off + wrapped.shape[1]] = np.tile(wrapped, (8, 1))

    # host-precomputed one-hot scatter matrices: S[p, cid*128 + v] = norm of the
    # edge in slot p of chunk cid targeting within-tile node v (0 for pad slots)
    bf = ml_dtypes.bfloat16
    S_full = np.zeros((n_cores, 128, NCHUNK * 128), dtype=bf)
    for c in range(n_cores):
        for t in range(NT):
            for half, nch_arr in ((0, nlo), (1, nhi)):
                c0 = chunk_id[(t, half, 0)]
                n_ = int(nch_arr[t])
                sl = slice(pad_off[t, half], pad_off[t, half] + n_ * 128)
                v = vpad[c, sl].astype(np.int64).reshape(n_, 128)   # [chunk, p]
                nval = npad[c, sl].reshape(n_, 128)
                for j in range(n_):
                    S_full[c, np.arange(128), (c0 + j) * 128 + v[j]] = \
                        nval[j].astype(bf)

    # exact per-core edge count per (tile, half) bucket -> num_idxs_reg
    bcnt = np.ascontiguousarray(
        cnt.reshape(n_cores, NT * 2).astype(np.int32))

    return dict(nlo=nlo, nhi=nhi, groups=groups, NCHUNK=NCHUNK, W_tot=W_tot,
                call_off=call_off, buckets=buckets, chunk_id=chunk_id,
                buf_pos=buf_pos, S_full=S_full, bcnt=bcnt), gidx


def build_nc(cfg, plan, b_out_val):
    nc = bacc.Bacc("TRN2", target_bir_lowering=False, debug=False,
                   num_devices=cfg.n_cores, num_swdge_queues=4,
                   dynamic_dma_scratch_size=32768)
    H, NT, NSH, L = cfg.H, cfg.NT, cfg.NSH, cfg.L
    NCHUNK = plan["NCHUNK"]
    nlo, nhi, groups = plan["nlo"], plan["nhi"], plan["groups"]
    call_off, chunk_id, buf_pos = plan["call_off"], plan["chunk_id"], plan["buf_pos"]
    buckets = plan["buckets"]
    maxlo = max(lo for lo, hi in call_off)
    maxhi = max(hi for lo, hi in call_off)
    RG = [list(range(cfg.n_cores))]
    last_rows = NSH - (NT - 1) * 128

    p_gidx = nc.declare_dram_parameter("gidx", [128, plan["W_tot"]], I16, isOutput=False)
    p_bcnt = nc.declare_dram_parameter("bcnt", [1, NT * 2], I32, isOutput=False)
    p_S = nc.declare_dram_parameter("S_full", [128, NCHUNK * 128], BF16, isOutput=False)
    p_xT = nc.declare_dram_parameter("xT", [cfg.IN, cfg.NSH_PAD], F32, isOutput=False)
    p_ident = nc.declare_dram_parameter("ident", [128, 128], BF16, isOutput=False)
    p_wemb = nc.declare_dram_parameter("wembT", [cfg.IN, H], F32, isOutput=False)
    p_bemb = nc.declare_dram_parameter("bemb", [1, H], F32, isOutput=False)
    p_wih_rz = nc.declare_dram_parameter("wih_rz", [H, L * 256], BF16, isOutput=False)
    p_wih_n = nc.declare_dram_parameter("wih_n", [H, L * 128], BF16, isOutput=False)
    p_whh_rz = nc.declare_dram_parameter("whh_rz", [H, L * 256], BF16, isOutput=False)
    p_whh_n = nc.declare_dram_parameter("whh_n", [H, L * 128], BF16, isOutput=False)
    p_ball = nc.declare_dram_parameter("ball", [1, L * 512], F32, isOutput=False)
    p_whid = nc.declare_dram_parameter("whidT", [H, cfg.HID], BF16, isOutput=False)
    p_bhid = nc.declare_dram_parameter("bhid", [cfg.HID, 1], F32, isOutput=False)
    p_wout = nc.declare_dram_parameter("woutT", [cfg.HID, 1], BF16, isOutput=False)
    p_ones = nc.declare_dram_parameter("ones1", [1, 128], F32, isOutput=False)
    p_out = nc.declare_dram_parameter("out", [NSH, 1], F32, isOutput=True)

    T1, T2 = cfg.T1, cfg.NSH - cfg.T1
    tabA1 = nc.dram_tensor("tabA1", [cfg.n_cores * T1, H], BF16, addr_space="Shared")
    tabA2 = nc.dram_tensor("tabA2", [cfg.n_cores * T2, H], BF16, addr_space="Shared")
    tabB1 = nc.dram_tensor("tabB1", [cfg.n_cores * T1, H], BF16, addr_space="Shared")
    tabB2 = nc.dram_tensor("tabB2", [cfg.n_cores * T2, H], BF16, addr_space="Shared")
    shard_out = nc.dram_tensor("shard_out", [NSH, H], BF16)

    with TileContext(nc) as tc:
        with (
            tc.tile_pool(name="consts", bufs=1) as cpool,
            tc.tile_pool(name="zpool", bufs=1) as zpool,
            tc.tile_pool(name="work", bufs=3) as wpool,
            tc.tile_pool(name="spool", bufs=2) as spool,
            tc.tile_pool(name="gw", bufs=2) as gw,
            tc.tile_pool(name="psA", bufs=2, space="PSUM") as ppool,
            tc.tile_pool(name="psB", bufs=2, space="PSUM") as qpool,
            tc.tile_pool(name="psT", bufs=2, space="PSUM") as tpool,
        ):
            def load_const(ap, dtype, tag):
                t = cpool.tile(list(ap.shape), dtype, tag=tag)
                nc.sync.dma_start(out=t[:], in_=ap[:])
                return t

            gidx_t = load_const(p_gidx, I16, "gidx")
            bcnt_t = load_const(p_bcnt, I32, "bcnt")
            xT_t = load_const(p_xT, F32, "xT")
            ident_t = load_const(p_ident, BF16, "ident")
            wemb_t = load_const(p_wemb, F32, "wemb")
            bemb_t = load_const(p_bemb, F32, "bemb")
            wih_rz_t = load_const(p_wih_rz, BF16, "wihrz")
            wih_n_t = load_const(p_wih_n, BF16, "wihn")
            whh_rz_t = load_const(p_whh_rz, BF16, "whhrz")
            whh_n_t = load_const(p_whh_n, BF16, "whhn")
            ball_t = load_const(p_ball, F32, "ball")
            whid_t = load_const(p_whid, BF16, "whid")
            bhid_t = load_const(p_bhid, F32, "bhid")
            wout_t = load_const(p_wout, BF16, "wout")
            ones_t = load_const(p_ones, F32, "ones")

            bcnt_reg = nc.gpsimd.alloc_register("bcnt_reg")
            zmax_t = zpool.tile([128, cfg.NSH_PAD], BF16, tag="zmax")
            h_shA = zpool.tile([128, cfg.NSH_PAD], BF16, tag="hshA")
            h_shB = zpool.tile([128, cfg.NSH_PAD], BF16, tag="hshB")
            h_bufs = [h_shA, h_shB]
            # fixed ping-pong gather buffers (memset once: -1-trimmed pad
            # slots keep stale-but-finite data, zeroed by the S matmul)
            glo_bufs = [zpool.tile([128, maxlo * 128], BF16, tag=f"glo{i}",
                                   name=f"glo{i}") for i in range(2)]
            ghi_bufs = [zpool.tile([128, maxhi * 128], BF16, tag=f"ghi{i}",
                                   name=f"ghi{i}") for i in range(2)]
            ones_f = zpool.tile([128, 128], F32, tag="onesf")
            nc.vector.memset(ones_f[:], 1.0)
            eps_f = zpool.tile([128, 8], F32, tag="epsf")
            nc.vector.memset(eps_f[:], EPS)
            for b in glo_bufs + ghi_bufs:
                nc.vector.memset(b[:], 0)

            def rows_of(t):
                return 128 if t < NT - 1 else last_rows

            def l2norm_bf(pre_t, out_ap):
                """l2-normalize pre_t [128,H] f32 -> bf16 out_ap."""
                sq = wpool.tile([128, H], F32, tag="sq")
                ss = wpool.tile([128, 1], F32, tag="ss")
                nc.scalar.activation(sq[:], pre_t[:], AF.Square, accum_out=ss[:])
                s1 = wpool.tile([128, 1], F32, tag="s1")
                nc.scalar.activation(s1[:], ss[:], AF.Sqrt)
                s2 = wpool.tile([128, 1], F32, tag="s2")
                nc.vector.tensor_tensor(s2[:], s1[:], eps_f[:, 0:1], ALU.max)
                rec = wpool.tile([128, 1], F32, tag="rec")
                nc.vector.reciprocal(rec[:], s2[:])
                nc.vector.tensor_tensor(out_ap, pre_t[:],
                                        rec[:].broadcast_to((128, H)), ALU.mult)

            # ================= EMBED =================
            for t in range(NT):
                ps = ppool.tile([128, H], F32, tag="acc")
                nc.tensor.matmul(ps[:], ones_t[:], bemb_t[:], start=True, stop=False)
                nc.tensor.matmul(ps[:], xT_t[:, t * 128:(t + 1) * 128], wemb_t[:],
                                 start=False, stop=True)
                h0f = wpool.tile([128, H], F32, tag="pref")
                nc.scalar.activation(h0f[:], ps[:], AF.Relu)
                hsl = h_bufs[0][:, t * 128:(t + 1) * 128]
                l2norm_bf(h0f, hsl)
                nc.vector.tensor_copy(zmax_t[:, t * 128:(t + 1) * 128], hsl)
                r = rows_of(t)
                nc.sync.dma_start(out=shard_out[t * 128: t * 128 + r, :],
                                  in_=h_bufs[0][:r, t * 128: t * 128 + H])
                if t == 27:
                    nc.gpsimd.collective_compute(
                        "AllGather", ALU.bypass, replica_groups=RG,
                        ins=[shard_out[0:T1, :]], outs=[tabA1[:]],
                    )
            nc.gpsimd.collective_compute(
                "AllGather", ALU.bypass, replica_groups=RG,
                ins=[shard_out[T1:NSH, :]], outs=[tabA2[:]],
            )

            # ================= GRU LAYERS =================
            for l in range(L):
                tp1 = tabA1 if l % 2 == 0 else tabB1
                tp2 = tabA2 if l % 2 == 0 else tabB2
                tc1 = tabB1 if l % 2 == 0 else tabA1
                tc2 = tabB2 if l % 2 == 0 else tabA2
                qrot = 0
                for g_, ts in enumerate(groups):
                    cid_g0 = chunk_id[(ts[0], 0, 0)]
                    gch = int(sum(nlo[t] + nhi[t] for t in ts))
                    Sg = spool.tile([128, gch * 128], BF16, tag="S")
                    nc.sync.dma_start(
                        out=Sg[:],
                        in_=p_S[:, cid_g0 * 128:(cid_g0 + gch) * 128])
                    glo = glo_bufs[g_ % 2]
                    ghi = ghi_bufs[g_ % 2]
                    for (t, half, off, n_b, p0) in buckets[g_]:
                        if n_b == 0:
                            continue
                        buf = glo if half == 0 else ghi
                        tab_src = tp1 if half == 0 else tp2
                        nc.gpsimd.reg_load(
                            bcnt_reg,
                            bcnt_t[0:1, 2 * t + half: 2 * t + half + 1])
                        nc.gpsimd.dma_gather(
                            buf[:, p0 * 128:(p0 + n_b) * 128]
                                .rearrange("p (c e) -> p c e", e=128),
                            tab_src[:],
                            gidx_t[:, off: off + 8 * n_b],
                            n_b * 128, bcnt_reg, H,
                            single_packet=cfg.single_packet,
                            queue_num=qrot % 4,
                        )
                        qrot += 1
                    h_prev = h_bufs[l % 2]
                    h_next = h_bufs[(l + 1) % 2]
                    NTS = len(ts)
                    td = [dict(t=t, base=t * 128) for t in ts]
                    # S0: hT transposes (PE) + copies (scalar, Copy table)
                    for i, d_ in enumerate(td):
                        tps = tpool.tile([128, H], BF16, tag="tps")
                        nc.tensor.transpose(
                            tps[:], h_prev[:, d_["base"]:d_["base"] + H],
                            ident_t[:])
                        d_["hT"] = gw.tile([128, H], BF16, tag=f"hT{i}",
                                           name=f"hT{i}")
                        nc.scalar.activation(d_["hT"][:], tps[:], AF.Copy)
                    # S1: scatter matmul chains + aggT copies
                    for i, d_ in enumerate(td):
                        t = d_["t"]
                        nch = int(nlo[t] + nhi[t])
                        sbase = chunk_id[(t, 0, 0)] - cid_g0
                        aggT_ps = ppool.tile([128, H], F32, tag="acc")
                        for k in range(nch):
                            if k < int(nlo[t]):
                                buf, bp = glo, buf_pos[(t, 0, k)]
                            else:
                                buf, bp = ghi, buf_pos[(t, 1, k - int(nlo[t]))]
                            nc.tensor.matmul(
                                aggT_ps[:],
                                buf[:, bp * 128:(bp + 1) * 128],
                                Sg[:, (sbase + k) * 128:(sbase + k + 1) * 128],
                                start=(k == 0), stop=(k == nch - 1),
                            )
                        d_["aggT"] = gw.tile([128, H], BF16, tag=f"aggT{i}",
                                             name=f"aggT{i}")
                        nc.scalar.activation(d_["aggT"][:], aggT_ps[:], AF.Copy)
                    # S2: gate PSUM bias init (scalar Copy) + gate matmuls (PE)
                    for i, d_ in enumerate(td):
                        gall = qpool.tile([128, 512], F32, tag=f"gall{i}",
                                          name=f"gall{i}", bufs=1)
                        d_["gall"] = gall
                        nc.tensor.matmul(gall[:, 0:256], ones_t[:],
                                         ball_t[:, l * 512:l * 512 + 256],
                                         start=True, stop=False)
                        nc.tensor.matmul(gall[:, 0:256], d_["aggT"][:],
                                         wih_rz_t[:, l * 256:(l + 1) * 256],
                                         start=False, stop=False)
                        nc.tensor.matmul(gall[:, 0:256], d_["hT"][:],
                                         whh_rz_t[:, l * 256:(l + 1) * 256],
                                         start=False, stop=True)
                        nc.tensor.matmul(gall[:, 256:384], ones_t[:],
                                         ball_t[:, l * 512 + 256:l * 512 + 384],
                                         start=True, stop=False)
                        nc.tensor.matmul(gall[:, 256:384], d_["aggT"][:],
                                         wih_n_t[:, l * 128:(l + 1) * 128],
                                         start=False, stop=True)
                        nc.tensor.matmul(gall[:, 384:512], ones_t[:],
                                         ball_t[:, l * 512 + 384:(l + 1) * 512],
                                         start=True, stop=False)
                        nc.tensor.matmul(gall[:, 384:512], d_["hT"][:],
                                         whh_n_t[:, l * 128:(l + 1) * 128],
                                         start=False, stop=True)
                    # S3: sigmoid batch (r, z gates)
                    for i, d_ in enumerate(td):
                        d_["rzt"] = gw.tile([128, 256], F32, tag=f"rzt{i}",
                                            name=f"rzt{i}")
                        nc.scalar.activation(d_["rzt"][:],
                                             d_["gall"][:, 0:256], AF.Sigmoid)
                    # S4: candidate pre-activation (DVE)
                    for i, d_ in enumerate(td):
                        t1 = wpool.tile([128, H], F32, tag="t1")
                        nc.vector.tensor_mul(t1[:], d_["rzt"][:, 0:128],
                                             d_["gall"][:, 384:512])
                        d_["t2"] = gw.tile([128, H], F32, tag=f"t2{i}",
                                           name=f"t2{i}")
                        nc.vector.tensor_add(d_["t2"][:], t1[:],
                                             d_["gall"][:, 256:384])
                    # S5: sigmoid batch #2: tanh(x) = 2*sigmoid(2x) - 1
                    for i, d_ in enumerate(td):
                        d_["sg2"] = gw.tile([128, H], F32, tag=f"sg2{i}",
                                            name=f"sg2{i}")
                        nc.scalar.activation(d_["sg2"][:], d_["t2"][:],
                                             AF.Sigmoid, scale=2.0)
                    # S6: GRU blend (DVE)
                    for i, d_ in enumerate(td):
                        hp_sl = h_prev[:, d_["base"]:d_["base"] + H]
                        ng = wpool.tile([128, H], F32, tag="ng")
                        nc.vector.scalar_tensor_tensor(
                            ng[:], d_["sg2"][:], 2.0, ones_f[:],
                            ALU.mult, ALU.subtract)
                        dd = wpool.tile([128, H], F32, tag="d")
                        nc.vector.tensor_sub(dd[:], hp_sl, ng[:])
                        e = wpool.tile([128, H], F32, tag="e")
                        nc.vector.tensor_mul(e[:], dd[:], d_["rzt"][:, 128:256])
                        d_["pre"] = gw.tile([128, H], F32, tag=f"pre{i}",
                                            name=f"pre{i}")
                        nc.vector.tensor_add(d_["pre"][:], e[:], ng[:])
                    # S7: squares batch (scalar) into grouped accum columns
                    ssg = gw.tile([128, NTS], F32, tag="ssg")
                    for i, d_ in enumerate(td):
                        sq = wpool.tile([128, H], F32, tag="sq")
                        nc.scalar.activation(sq[:], d_["pre"][:], AF.Square,
                                             accum_out=ssg[:, i:i + 1])
                    # S8: batched sqrt + guard + reciprocal
                    s1g = gw.tile([128, NTS], F32, tag="s1g")
                    nc.scalar.activation(s1g[:], ssg[:], AF.Sqrt)
                    s2g = gw.tile([128, NTS], F32, tag="s2g")
                    nc.vector.tensor_tensor(s2g[:], s1g[:], eps_f[:, 0:NTS],
                                            ALU.max)
                    recg = gw.tile([128, NTS], F32, tag="recg")
                    nc.vector.reciprocal(recg[:], s2g[:])
                    # S9: normalize, layer-max, shard write
                    for i, d_ in enumerate(td):
                        base = d_["base"]
                        hn_sl = h_next[:, base:base + H]
                        nc.vector.tensor_tensor(
                            hn_sl, d_["pre"][:],
                            recg[:, i:i + 1].broadcast_to((128, H)), ALU.mult)
                        nc.vector.tensor_max(zmax_t[:, base:base + 128],
                                             zmax_t[:, base:base + 128], hn_sl)
                        if l < L - 1:
                            r = rows_of(d_["t"])
                            nc.sync.dma_start(
                                out=shard_out[base: base + r, :],
                                in_=h_next[:r, base:base + H])
                    if l < L - 1 and ts[-1] == 27:
                        nc.gpsimd.collective_compute(
                            "AllGather", ALU.bypass, replica_groups=RG,
                            ins=[shard_out[0:T1, :]], outs=[tc1[:]],
                        )
                if l < L - 1:
                    nc.gpsimd.collective_compute(
                        "AllGather", ALU.bypass, replica_groups=RG,
                        ins=[shard_out[T1:NSH, :]], outs=[tc2[:]],
                    )

            # ================= DECODER =================
            for t in range(NT):
                r = rows_of(t)
                base = t * 128
                tps = tpool.tile([128, H], BF16, tag="tps")
                nc.tensor.transpose(tps[:], zmax_t[:, base:base + 128], ident_t[:])
                zT = wpool.tile([128, H], BF16, tag="zT")
                nc.vector.tensor_copy(zT[:], tps[:])
                galld = qpool.tile([128, 512], F32, tag="gall0",
                                   name="galld", bufs=1)
                hid_ps = galld[0:cfg.HID, 0:128]
                nc.tensor.matmul(hid_ps, whid_t[:], zT[:], start=True, stop=True)
                hid = wpool.tile([cfg.HID, 128], BF16, tag="hid")
                nc.scalar.activation(hid[:], hid_ps, AF.Relu, bias=bhid_t[:])
                o_ps = galld[0:1, 256:384]
                nc.tensor.matmul(o_ps, wout_t[:], hid[:], start=True, stop=True)
                o_sb = wpool.tile([1, 128], F32, tag="osb")
                nc.scalar.activation(o_sb[:], o_ps, AF.Copy, bias=float(b_out_val))
                nc.sync.dma_start(out=p_out[base: base + r, :], in_=o_sb[:1, :r])
    nc.compile()
    return nc


def make_in_maps(cfg, inputs, plan, gidx):
    bf = ml_dtypes.bfloat16
    L, H, NSH = cfg.L, cfg.H, cfg.NSH
    x = np.asarray(inputs["x"], np.float32)
    w_ih = np.asarray(inputs["w_ih"], np.float32)
    w_hh = np.asarray(inputs["w_hh"], np.float32)
    b_ih = np.asarray(inputs["b_ih"], np.float32)
    b_hh = np.asarray(inputs["b_hh"], np.float32)

    wih_rz = np.concatenate([w_ih[l, :256, :].T for l in range(L)], axis=1)
    wih_n = np.concatenate([w_ih[l, 256:384, :].T for l in range(L)], axis=1)
    whh_rz = np.concatenate([w_hh[l, :256, :].T for l in range(L)], axis=1)
    whh_n = np.concatenate([w_hh[l, 256:384, :].T for l in range(L)], axis=1)
    ball = np.concatenate(
        [np.concatenate([b_ih[l, :256] + b_hh[l, :256],
                         b_ih[l, 256:384], b_hh[l, 256:384]])
         for l in range(L)])[None, :]

    common = {
        "ident": np.eye(128, dtype=bf),
        "wembT": np.ascontiguousarray(np.asarray(inputs["W_embed"], np.float32).T),
        "bemb": np.asarray(inputs["b_embed"], np.float32)[None, :],
        "wih_rz": np.ascontiguousarray(wih_rz, dtype=bf),
        "wih_n": np.ascontiguousarray(wih_n, dtype=bf),
        "whh_rz": np.ascontiguousarray(whh_rz, dtype=bf),
        "whh_n": np.ascontiguousarray(whh_n, dtype=bf),
        "ball": np.ascontiguousarray(ball),
        "whidT": np.ascontiguousarray(np.asarray(inputs["W_hid"], np.float32).T,
                                      dtype=bf),
        "bhid": np.asarray(inputs["b_hid"], np.float32)[:, None],
        "woutT": np.ascontiguousarray(np.asarray(inputs["W_out"], np.float32).T,
                                      dtype=bf),
        "ones1": np.ones((1, 128), np.float32),
    }
    in_maps = []
    for c in range(cfg.n_cores):
        xT = np.zeros((cfg.IN, cfg.NSH_PAD), np.float32)
        xT[:, :NSH] = x[c * NSH:(c + 1) * NSH, :].T
        m = dict(common)
        m["xT"] = xT
        m["gidx"] = gidx[c]
        m["S_full"] = plan["S_full"][c]
        m["bcnt"] = plan["bcnt"][c][None, :]
        in_maps.append(m)
    return in_maps


def kernel(**inputs):
    cfg = Cfg()
    plan, gidx = build_plan(cfg, np.asarray(inputs["edge_idx"]))
    nc = build_nc(cfg, plan, float(np.asarray(inputs["b_out"]).ravel()[0]))
    in_maps = make_in_maps(cfg, inputs, plan, gidx)
    res = run_bass_kernel_spmd(nc, in_maps, list(range(cfg.n_cores)))
    out = np.concatenate([res.results[c]["out"] for c in range(cfg.n_cores)], axis=0)
    return out.astype(np.float32)

